# revision 5
# baseline (speedup 1.0000x reference)
"""Kuramoto-Sivashinsky RHS on 8 Trainium2 NeuronCores (Bass/Tile).

Math: for each row u (length 64), with k = 2*pi*fftfreq(64, 0.5):
    u_x  = Re(ifft(1j*k*fft(u)))       = u @ D1^T   (D1 real 64x64)
    lin  = Re(ifft((k^2-k^4)*fft(u)))  = u @ L^T    (L  real 64x64)
    out  = -0.5*u*u_x + lin
The spectral chain is a fixed real linear operator per row, so the kernel is
two small matmuls + a pointwise multiply-add. Data parallel over rows:
1048576 rows -> 8 shards of 131072.

Per-core pipeline (chunk = 512 rows = [128,256] tile, partition p holds rows
4p..4p+3 of the chunk):
  1. DMA in  (contiguous, 1KB/partition)
  2. 2x PE transpose [128,128] -> psum uT (f32r, spatial-major)
  3. ACT copy psum->sbuf
  4. 4x f32r matmul  uT_c[64,128]^T @ Wpad[64,256] -> psum (natural layout;
     Wpad = [-0.5*D1^T | L^T | 0pad] ; N=256 padding hits the f32r
     1-cycle/row fast path)
  5. DVE: prod = u * psum_ux ; res = prod + psum_lin
  6. DMA out (contiguous)
"""

import sys

import numpy as np

sys.path.insert(0, "/opt/trn_rl_repo")

N_CORES = 8
B_TOTAL = 1048576
N = 64
B_SHARD = B_TOTAL // N_CORES  # 131072
CH_ROWS = 512                  # rows per chunk
N_CHUNKS = B_SHARD // CH_ROWS  # 256


def make_weights():
    """Wpad [64,256] f32: cols 0:64 = -0.5*D1^T, 64:128 = L^T, rest zero."""
    k = 2.0 * np.pi * np.fft.fftfreq(N, d=0.5)
    F = np.fft.fft(np.eye(N), axis=0)
    D1 = np.real(np.fft.ifft(1j * k[:, None] * F, axis=0))
    L = np.real(np.fft.ifft((k**2 - k**4)[:, None] * F, axis=0))
    W = np.zeros((N, 256), np.float64)
    W[:, 0:64] = -0.5 * D1.T
    W[:, 64:128] = L.T
    W = W.astype(np.float32)
    return np.vstack([W, W])  # [128,256]: same W at partitions 0-63 and 64-127


def build(b_shard=B_SHARD, ch_rows=CH_ROWS):
    """Build + compile the Bass program for one shard of b_shard rows."""
    from contextlib import ExitStack

    import concourse.bacc as bacc
    import concourse.mybir as mybir
    import concourse.tile as tile

    f32 = mybir.dt.float32
    f32r = mybir.dt.float32r
    n_chunks = b_shard // ch_rows
    cpc = ch_rows // 128  # column-blocks of 64 per chunk (4 for 512 rows)

    nc = bacc.Bacc("TRN2", target_bir_lowering=False, debug=False)
    u = nc.dram_tensor("u", [b_shard, N], f32, kind="ExternalInput").ap()
    w = nc.dram_tensor("w", [128, 256], f32, kind="ExternalInput").ap()
    ident = nc.dram_tensor("ident", [128, 128], f32, kind="ExternalInput").ap()
    out = nc.dram_tensor("out", [b_shard, N], f32, kind="ExternalOutput").ap()

    uv = u.rearrange("(i p c) m -> i p (c m)", p=128, c=cpc)
    ov = out.rearrange("(i p c) m -> i p (c m)", p=128, c=cpc)

    with tile.TileContext(nc) as tc, ExitStack() as ctx:
        cpool = ctx.enter_context(tc.tile_pool(name="const", bufs=1))
        wt = cpool.tile([128, 256], f32, tag="w")
        nc.sync.dma_start(wt[:], w)
        wtr = cpool.tile([128, 256], f32r, tag="wr")
        nc.scalar.copy(wtr[:], wt[:])
        it = cpool.tile([128, 128], f32, tag="ident")
        nc.sync.dma_start(it[:], ident)

        inp = ctx.enter_context(tc.tile_pool(name="inp", bufs=4))
        utp = ctx.enter_context(tc.tile_pool(name="utp", bufs=3))
        pvp = ctx.enter_context(tc.tile_pool(name="pv", bufs=3))
        rsp = ctx.enter_context(tc.tile_pool(name="rs", bufs=3))
        psT = ctx.enter_context(tc.tile_pool(name="psT", bufs=2, space="PSUM"))
        psN = ctx.enter_context(tc.tile_pool(name="psN", bufs=1, space="PSUM"))

        for i in range(n_chunks):
            U = inp.tile([128, 64 * cpc], f32, tag="U")
            nc.sync.dma_start(U[:], uv[i])

            # transpose: U[:,128c:128c+128] -> pT[:,128c:128c+128]
            pT = psT.tile([128, 64 * cpc], f32, tag="pT")
            for c2 in range(cpc // 2):
                nc.tensor.matmul(
                    pT[:, 128 * c2 : 128 * c2 + 128],
                    U[:, 128 * c2 : 128 * c2 + 128],
                    it[:],
                    is_transpose=True,
                    start=True,
                    stop=True,
                )

            uT = utp.tile([128, 64 * cpc], f32r, tag="uT")
            nc.scalar.copy(uT[:], pT[:])

            # natural-layout matmuls: one psum BANK (512 f32) per 128-row
            # block; one start/stop group per bank (two groups in one bank
            # crash the runtime)
            nat = psN.tile([128, 512 * cpc], f32, tag="nat")
            for c in range(cpc):
                lhsT = uT[
                    64 * (c % 2) : 64 * (c % 2) + 64,
                    128 * (c // 2) : 128 * (c // 2) + 128,
                ]
                dst = nat[:, 512 * c : 512 * c + 256]
                rhsw = wtr[64 * (c % 2) : 64 * (c % 2) + 64, :]
                nc.tensor.matmul(dst, lhsT, rhsw, start=True, stop=True)

            prod = pvp.tile([128, 64 * cpc], f32, tag="prod")
            res = rsp.tile([128, 64 * cpc], f32, tag="res")
            natv = nat[:].rearrange("p (b n) -> p b n", b=cpc)
            u3 = U[:].rearrange("p (b n) -> p b n", b=cpc)
            p3 = prod[:].rearrange("p (b n) -> p b n", b=cpc)
            r3 = res[:].rearrange("p (b n) -> p b n", b=cpc)
            nc.vector.tensor_mul(p3, u3, natv[:, :, 0:64])
            nc.vector.tensor_add(r3, p3, natv[:, :, 64:128])

            nc.sync.dma_start(ov[i], res[:])

    nc.compile()
    return nc


_CACHE = {}


def _get_nc():
    if "nc" not in _CACHE:
        _CACHE["nc"] = build()
    return _CACHE["nc"]


def kernel(t, u):
    from concourse.bass_utils import run_bass_kernel_spmd

    u = np.ascontiguousarray(np.asarray(u, dtype=np.float32))
    assert u.shape == (B_TOTAL, N)
    nc = _get_nc()
    W = make_weights()
    I = np.eye(128, dtype=np.float32)
    shards = u.reshape(N_CORES, B_SHARD, N)
    in_maps = [{"u": shards[i], "w": W, "ident": I} for i in range(N_CORES)]
    res = run_bass_kernel_spmd(nc, in_maps, list(range(N_CORES)))
    return np.concatenate([res.results[i]["out"] for i in range(N_CORES)], axis=0)
